# revision 1
# baseline (speedup 1.0000x reference)
"""ChildSum TreeLSTM on TRN2, 8-core SPMD Bass/Tile kernel — v2.

v2 changes vs v1:
- bias folded into the matmul via a constant-1 row of x (row 300), so ACT
  calls need no per-Mtile bias and can span both H-Mtiles at once
- Mtile-in-columns layout: every elementwise tile is [128, 2*cols] with the
  two H-halves side by side -> half the ACT/DVE instruction count
- fi computed as its own small matmul over parent columns (not folded into
  the fh matmul over child columns): PE -20us, DVE +fi-broadcast-add
- all internal-level chunks <= 256 parents so PSUM tags fit in 8 banks
- i*u and f*cc multiplies moved to GPSIMD (SBUF-only operands), keeping DVE
  for reduces and PSUM-reading adds
"""

import numpy as np

D = 300
DR = 301        # +1 constant-1 bias row
H = 256
KB = 4
N_CORES = 8
SPLIT_LEVEL = 3
PRECISE_LMAX = 4   # levels <= this run their matmuls in true fp32
P = 128
XCH = [(0, 128), (128, 256), (256, DR)]   # x contraction chunks


def levels_of(n, k=KB):
    levels, start, size = [], 0, 1
    while start < n:
        end = min(start + size, n)
        levels.append((start, end))
        start, size = end, size * k
    return levels


def level_starts(lmax):
    return [(4**l - 1) // 3 for l in range(lmax + 2)]


def ref_np(inputs, ix_w, ix_b, ih_w, ih_b, ux_w, ux_b, uh_w, uh_b,
           fi_w, fi_b, fh_w, fh_b):
    n = inputs.shape[0]
    ix = inputs @ ix_w.T + ix_b
    ux = inputs @ ux_w.T + ux_b
    fi = inputs @ fi_w.T + fi_b
    h = np.zeros((n, H), np.float32)
    c = np.zeros((n, H), np.float32)
    for (s, e) in reversed(levels_of(n)):
        node = np.arange(s, e)
        cidx = node[:, None] * KB + 1 + np.arange(KB)[None, :]
        valid = cidx < n
        cidx = np.where(valid, cidx, 0)
        m = valid[..., None].astype(np.float32)
        hc = h[cidx] * m
        cc = c[cidx] * m
        h_sum = hc.sum(axis=1)
        f = 1.0 / (1.0 + np.exp(-(fi[s:e][:, None, :] + hc @ fh_w.T + fh_b)))
        fc = (f * cc).sum(axis=1)
        i = 1.0 / (1.0 + np.exp(-(ix[s:e] + h_sum @ ih_w.T + ih_b)))
        u = np.tanh(ux[s:e] + h_sum @ uh_w.T + uh_b)
        c_new = i * u + fc
        h[s:e] = np.tanh(c_new)
        c[s:e] = c_new
    return h[0], c[0]


def _layout(n):
    lv = levels_of(n)
    lmax = len(lv) - 1
    S = level_starts(lmax)
    m = {l: (4**l) // N_CORES for l in range(SPLIT_LEVEL, lmax + 1)}
    offs, o = {}, 0
    for l in range(SPLIT_LEVEL, lmax + 1):
        offs[l] = o
        o += m[l]
    off_top = o
    n_top = S[SPLIT_LEVEL]
    # +2 trailing columns: this core's two level-2 nodes (slots 5+2g, 6+2g)
    return lmax, S, m, offs, off_top, off_top + n_top + 2


def prep_inputs(n, inputs, ix_w, ix_b, ih_w, ih_b, ux_w, ux_b, uh_w, uh_b,
                fi_w, fi_b, fh_w, fh_b):
    lmax, S, m, offs, off_top, ncols = _layout(n)
    assert lmax >= SPLIT_LEVEL + 1
    n_top = S[SPLIT_LEVEL]

    v = np.linalg.lstsq(ux_w.astype(np.float64),
                        -(ux_b + uh_b).astype(np.float64), rcond=None)[0]
    v = np.concatenate([v.astype(np.float32), [1.0]])      # bias row = 1

    xT = inputs.T.astype(np.float32)

    wproj = np.empty((DR, 2 * H), np.float32)
    wproj[:D, :H] = ix_w.T
    wproj[:D, H:] = ux_w.T
    wproj[D, :H] = ix_b + ih_b
    wproj[D, H:] = ux_b + uh_b
    whh = np.concatenate([ih_w.T, uh_w.T], axis=1)          # [256, 512]
    wfi = np.empty((DR, H), np.float32)
    wfi[:D] = fi_w.T
    wfi[D] = fi_b + fh_b
    wfh = np.ascontiguousarray(fh_w.T)                      # [256, 256]

    in_maps = []
    for g in range(N_CORES):
        xg = np.empty((DR, ncols), np.float32)
        xg[D, :] = 1.0
        for l in range(SPLIT_LEVEL, lmax + 1):
            s0 = S[l] + m[l] * g
            cnt = m[l]
            n_real = min(max(n - s0, 0), cnt)
            if n_real > 0:
                xg[:D, offs[l]:offs[l] + n_real] = xT[:, s0:s0 + n_real]
            if n_real < cnt:
                xg[:, offs[l] + n_real:offs[l] + cnt] = v[:, None]
        xg[:D, off_top:off_top + n_top] = xT[:, :n_top]
        my2 = 5 + 2 * g                     # first of this core's L2 nodes
        xg[:D, off_top + n_top:off_top + n_top + 2] = xT[:, my2:my2 + 2]
        in_maps.append({"xT": xg, "wproj": wproj, "whh": whh,
                       "wfi": wfi, "wfh": wfh})
    return in_maps, dict(lmax=lmax, m=m, offs=offs, off_top=off_top,
                         ncols=ncols)


def build_program(n, debug=False, timing=False, leaf_c_pool=False,
                  fcc_pool=True):
    import concourse.bass as bass
    import concourse.tile as tile
    from concourse import bacc, mybir

    f32 = mybir.dt.float32
    f32r = mybir.dt.float32r
    AF = mybir.ActivationFunctionType
    AX = mybir.AxisListType

    lmax, S, m, offs, off_top, ncols = _layout(n)
    m_leaf = m[lmax]
    LEAF_CHUNK = min(m_leaf, 1024)
    n_chunks = m_leaf // LEAF_CHUNK
    LPC = 256                        # max parents per internal-level call

    nc = bacc.Bacc("TRN2", target_bir_lowering=False, debug=debug,
                   num_devices=N_CORES)

    xT_d = nc.dram_tensor("xT", [DR, ncols], f32r, kind="ExternalInput")
    wproj_d = nc.dram_tensor("wproj", [DR, 2 * H], f32r, kind="ExternalInput")
    whh_d = nc.dram_tensor("whh", [H, 2 * H], f32r, kind="ExternalInput")
    wfi_d = nc.dram_tensor("wfi", [DR, H], f32r, kind="ExternalInput")
    wfh_d = nc.dram_tensor("wfh", [H, H], f32r, kind="ExternalInput")
    h0_d = nc.dram_tensor("h0", [P, 2], f32, kind="ExternalOutput")
    c0_d = nc.dram_tensor("c0", [P, 2], f32, kind="ExternalOutput")

    import concourse.bass as _bass

    def b2(t):                      # view [128, 2*cols] as [128, 2, cols]
        ap = t if isinstance(t, _bass.AP) else t[:]
        return ap.rearrange("p (b c) -> p b c", b=2)

    with tile.TileContext(nc) as tc:
        import contextlib
        with contextlib.ExitStack() as stack:
            wpool = stack.enter_context(tc.tile_pool(name="w", bufs=1))
            state = stack.enter_context(tc.tile_pool(name="state", bufs=1))
            leafp = stack.enter_context(tc.tile_pool(name="leafhc", bufs=3))
            xpool = stack.enter_context(tc.tile_pool(name="x", bufs=4))
            work = stack.enter_context(tc.tile_pool(name="work", bufs=2))
            psum = stack.enter_context(
                tc.tile_pool(name="psum", bufs=1, space="PSUM"))
            dram = stack.enter_context(
                tc.tile_pool(name="dram", bufs=1, space="DRAM"))

            # --- weights (lhsT chunks along the contraction dim) ---
            wproj = []
            for k, (r0, r1) in enumerate(XCH):
                t = wpool.tile([r1 - r0, 2 * H], f32r, name=f"wproj{k}")
                nc.gpsimd.dma_start(t[:], wproj_d[r0:r1, :])
                wproj.append(t)
            whh = [wpool.tile([P, 2 * H], f32r, name=f"whh{k}")
                   for k in range(2)]
            for k in range(2):
                nc.gpsimd.dma_start(whh[k][:], whh_d[k * P:(k + 1) * P, :])
            wfi = []
            for k, (r0, r1) in enumerate(XCH):
                t = wpool.tile([r1 - r0, H], f32r, name=f"wfi{k}")
                nc.gpsimd.dma_start(t[:], wfi_d[r0:r1, :])
                wfi.append(t)
            wfh = [wpool.tile([P, H], f32r, name=f"wfh{k}") for k in range(2)]
            for k in range(2):
                nc.gpsimd.dma_start(wfh[k][:], wfh_d[k * P:(k + 1) * P, :])

            # --- per-level state, Mtile-in-columns: [128, 2*m_l] ---
            hst, cst = {}, {}
            for l in range(SPLIT_LEVEL, lmax):
                hdt = f32 if l <= PRECISE_LMAX else f32r
                hst[l] = state.tile([P, 2 * m[l]], hdt, name=f"h{l}")
                cst[l] = state.tile([P, 2 * m[l]], f32, name=f"c{l}")
            top_cols = {1: 8, 0: 2}
            for l in range(SPLIT_LEVEL - 2, -1, -1):
                cnt = top_cols[l]
                hst[l] = state.tile([P, 2 * cnt], f32, name=f"h{l}")
                cst[l] = state.tile([P, 2 * cnt], f32, name=f"c{l}")
                if l == 1:
                    nc.vector.memset(b2(hst[l])[:, :, 4:8], 0.0)
                    nc.vector.memset(b2(cst[l])[:, :, 4:8], 0.0)
            h2l = state.tile([P, 2 * 2], f32, name="h2l")
            c2l = state.tile([P, 2 * 2], f32, name="c2l")
            h2g = state.tile([P, 2 * 16], f32, name="h2g")
            c2g = state.tile([P, 2 * 16], f32, name="c2g")

            def load_x(col0, cols, tag):
                xt = []
                for k, (r0, r1) in enumerate(XCH):
                    t = xpool.tile([r1 - r0, cols], f32r, name=f"x{tag}_{k}",
                                   tag=f"x{k}")
                    nc.sync.dma_start(t[:], xT_d[r0:r1, col0:col0 + cols])
                    xt.append(t)
                return xt

            def leaf_chunk(j, h8, c8):
                col0 = offs[lmax] + j * LEAF_CHUNK
                for s in range(0, LEAF_CHUNK, 512):
                    sub = min(512, LEAF_CHUNK - s)
                    xt = load_x(col0 + s, sub, f"lf{j}_{s}")
                    pi = psum.tile([P, 1024], f32, name=f"pi{j}_{s}",
                                   tag="i")
                    pu = psum.tile([P, 1024], f32, name=f"pu{j}_{s}",
                                   tag="u")
                    for mt in range(2):
                        for k in range(3):
                            nc.tensor.matmul(
                                pi[:, mt * 512:mt * 512 + sub],
                                wproj[k][:, mt * P:(mt + 1) * P], xt[k][:],
                                start=(k == 0), stop=(k == 2))
                        for k in range(3):
                            nc.tensor.matmul(
                                pu[:, mt * 512:mt * 512 + sub],
                                wproj[k][:, H + mt * P:H + (mt + 1) * P],
                                xt[k][:], start=(k == 0), stop=(k == 2))
                    it = work.tile([P, 2 * sub], f32, name=f"il{j}_{s}",
                                   tag="i")
                    ut = work.tile([P, 2 * sub], f32, name=f"ul{j}_{s}",
                                   tag="u")
                    pvi = b2(pi)[:, :, :sub]
                    pvu = b2(pu)[:, :, :sub]
                    nc.scalar.activation(b2(it), pvi, AF.Sigmoid)
                    nc.scalar.activation(b2(ut), pvu, AF.Tanh)
                    csl = b2(c8)[:, :, s:s + sub]
                    eng = nc.gpsimd if leaf_c_pool else nc.vector
                    eng.tensor_mul(csl, b2(it), b2(ut))
                nc.scalar.activation(b2(h8), b2(c8), AF.Tanh)

            def emit_fi(L, xt, tag, f32mode, on_act=True):
                W = (lambda t: t.bitcast(f32)) if f32mode else (lambda t: t)
                pfi = psum.tile([P, 2 * L], f32, name=f"pfi{tag}", tag="lp",
                                bufs=4)
                for mt in range(2):
                    for k in range(3):
                        nc.tensor.matmul(
                            pfi[:, mt * L:(mt + 1) * L],
                            W(wfi[k])[:, mt * P:(mt + 1) * P], W(xt[k])[:],
                            start=(k == 0), stop=(k == 2))
                fis = work.tile([P, 2 * L], f32, name=f"fis{tag}",
                                tag=f"fi{tag}" if 2 * L <= 512 else "fi",
                                bufs=1 if 2 * L <= 512 else None)
                if on_act:
                    nc.scalar.copy(fis[:], pfi[:])
                else:
                    nc.vector.tensor_copy(fis[:], pfi[:])
                return fis

            def level_chunk(L, x_col0, h_ch, c_ch, mch, ch0, h_out, c_out,
                            mout, oc0, tag, f32mode=False, xt=None,
                            fis=None, small=False):
                """L parents; children at cols [ch0, ch0+4L) of each Mtile
                block of h_ch/c_ch (block stride mch).  Output written at
                cols [oc0, oc0+L) of each block of h_out/c_out (stride mout).
                """
                W = (lambda t: t.bitcast(f32)) if f32mode else (lambda t: t)
                mdt = f32 if f32mode else f32r
                if xt is None:
                    xt = load_x(x_col0, L, tag)
                hch_b = h_ch[:].rearrange("p (b c) -> p b c", b=2)
                cch_b = c_ch[:].rearrange("p (b c) -> p b c", b=2)

                if fis is None:
                    fis = emit_fi(L, xt, tag, f32mode, on_act=False)

                # h_sum over 4 children (one 4D reduce)
                hs = work.tile([P, 2 * L], mdt, name=f"hs{tag}", tag="hs")
                with nc.allow_low_precision(reason="f32r round of f32 acc"):
                    nc.vector.reduce_sum(
                        b2(hs),
                        hch_b.bitcast(f32)[:, :, ch0:ch0 + 4 * L]
                        .rearrange("p b (l k) -> p b l k", k=4),
                        axis=AX.X)

                # i/u pre-activations.  Bulk levels: single-bank [P, 2L]
                # tiles (lp tag), h_sum part first so each (gate, mt) group
                # opens late and closes fast (one open group per bank).
                # Small levels: the leaf i/u tags are free, so use bank-
                # aligned [P, 1024] tiles and put the x part FIRST -- PE can
                # start it before h_sum is ready.
                if small:
                    pi = psum.tile([P, 1024], f32, name=f"pi{tag}", tag="i")
                    pu = psum.tile([P, 1024], f32, name=f"pu{tag}", tag="u")
                    stp = 512
                else:
                    pi = psum.tile([P, 2 * L], f32, name=f"pi{tag}", tag="lp",
                                   bufs=4)
                    pu = psum.tile([P, 2 * L], f32, name=f"pu{tag}", tag="lp",
                                   bufs=4)
                    stp = L
                def iu_mms():
                    for pt, base in ((pi, 0), (pu, H)):
                        for mt in range(2):
                            parts = [
                                ("h", [(k, W(whh[k]), hs[:, k * L:(k + 1) * L])
                                       for k in range(2)]),
                                ("x", [(k, W(wproj[k]), W(xt[k])[:])
                                       for k in range(3)]),
                            ]
                            if small:
                                parts.reverse()
                            first = True
                            for pi_, grp in parts:
                                last_part = pi_ == parts[-1][0]
                                for idx, (k, w, rhs) in enumerate(grp):
                                    nc.tensor.matmul(
                                        pt[:, mt * stp:mt * stp + L],
                                        w[:, base + mt * P:base + (mt + 1) * P],
                                        rhs, start=first,
                                        stop=(last_part and
                                              idx == len(grp) - 1))
                                    first = False

                # forget path over children: psum subs of 256 (single-bank
                # lp slots), fpre/f/fcc grouped in [P, 2*512] pair tiles so
                # ACT/Pool/DVE get half the calls
                fc = work.tile([P, 2 * L], f32, name=f"fc{tag}", tag="fc")
                subsz = 256
                fsteps = []
                for s in range(0, 4 * L, subsz):
                    sub = min(subsz, 4 * L - s)
                    p0, np_ = s // 4, sub // 4
                    pf = psum.tile([P, 2 * sub], f32, name=f"pf{tag}{s}",
                                   tag="lp", bufs=4)
                    for mt in range(2):
                        for k in range(2):
                            nc.tensor.matmul(
                                pf[:, mt * sub:(mt + 1) * sub],
                                W(wfh[k])[:, mt * P:(mt + 1) * P],
                                W(hch_b)[:, k, ch0 + s:ch0 + s + sub],
                                start=(k == 0), stop=(k == 1))
                    fsteps.append((s, sub, p0, np_, pf))
                iu_mms()
                # group psum subs into pairs sharing one SBUF tile
                pairs = [fsteps[i:i + 2] for i in range(0, len(fsteps), 2)]
                for gi, grp in enumerate(pairs):
                    gw = sum(x[1] for x in grp)          # children in group
                    gp = grp[0][2]                        # first parent
                    gnp = gw // 4
                    fpre = work.tile([P, 2 * gw], f32, name=f"fp{tag}{gi}",
                                     tag="fpre")
                    fv = fpre[:].rearrange("p (b c) -> p b c", b=2)
                    for (s, sub, p0, np_, pf) in grp:
                        o = s - grp[0][0]
                        firep = (fis[:].rearrange("p (b c) -> p b c", b=2)
                                 [:, :, p0:p0 + np_].unsqueeze(3)
                                 .broadcast_to([P, 2, np_, 4]))
                        nc.vector.tensor_add(
                            fv[:, :, o:o + sub]
                            .rearrange("p b (l k) -> p b l k", k=4),
                            pf[:].rearrange("p (b l k) -> p b l k", b=2, k=4),
                            firep)
                    ft = work.tile([P, 2 * gw], f32, name=f"f{tag}{gi}",
                                   tag="f")
                    nc.scalar.activation(ft[:], fpre[:], AF.Sigmoid)
                    fcc = work.tile([P, 2 * gw], f32, name=f"fx{tag}{gi}",
                                    tag="fcc")
                    eng = nc.vector if small else (
                        nc.gpsimd if fcc_pool else nc.vector)
                    eng.tensor_mul(
                        b2(fcc), b2(ft),
                        cch_b[:, :, ch0 + grp[0][0]:ch0 + grp[0][0] + gw])
                    nc.vector.reduce_sum(
                        fc[:].rearrange("p (b c) -> p b c", b=2)
                        [:, :, gp:gp + gnp],
                        fcc[:].rearrange("p (b l k) -> p b l k", b=2, k=4),
                        axis=AX.X)
                it = work.tile([P, 2 * L], f32, name=f"i{tag}", tag="i")
                ut = work.tile([P, 2 * L], f32, name=f"u{tag}", tag="u")
                if small:
                    nc.scalar.activation(b2(it), b2(pi)[:, :, :L], AF.Sigmoid)
                    nc.scalar.activation(b2(ut), b2(pu)[:, :, :L], AF.Tanh)
                else:
                    nc.scalar.activation(it[:], pi[:], AF.Sigmoid)
                    nc.scalar.activation(ut[:], pu[:], AF.Tanh)
                tmp = work.tile([P, 2 * L], f32, name=f"t{tag}", tag="tmp")
                (nc.vector if small else nc.gpsimd).tensor_mul(
                    tmp[:], it[:], ut[:])
                hob = h_out[:].rearrange("p (b c) -> p b c", b=2)
                cob = c_out[:].rearrange("p (b c) -> p b c", b=2)
                csl = cob[:, :, oc0:oc0 + L]
                nc.vector.tensor_add(csl, b2(tmp), b2(fc))
                nc.scalar.activation(hob[:, :, oc0:oc0 + L], csl, AF.Tanh)

            # ---------------- main flow ----------------
            # prefetch x for all small levels (<= lmax-3) and the top nodes
            n_small = offs[lmax - 2]
            n_top = S[SPLIT_LEVEL]
            xsmall, xtop = [], []
            for k, (r0, r1) in enumerate(XCH):
                t = state.tile([r1 - r0, n_small], f32r, name=f"xs{k}")
                nc.gpsimd.dma_start(t[:], xT_d[r0:r1, 0:n_small])
                xsmall.append(t)
                t2 = state.tile([r1 - r0, n_top + 2], f32r, name=f"xtp{k}")
                nc.gpsimd.dma_start(
                    t2[:], xT_d[r0:r1, off_top:off_top + n_top + 2])
                xtop.append(t2)
            def emit_hoisted_fi():
                for l in range(lmax - 3, SPLIT_LEVEL - 1, -1):
                    pre = [t[:, offs[l]:offs[l] + m[l]] for t in xsmall]
                    fis_pre[l] = emit_fi(m[l], pre, f"h{l}",
                                         l <= PRECISE_LMAX)
                pre = [t[:, n_top:n_top + 2] for t in xtop]
                fis_pre[("L2", 0)] = emit_fi(2, pre, "hL2", True)
                for l in range(SPLIT_LEVEL - 2, -1, -1):
                    cnt = 4**l if l > 0 else 2
                    x0 = (4**l - 1) // 3
                    pre = [t[:, x0:x0 + cnt] for t in xtop]
                    fis_pre[("T", l)] = emit_fi(cnt, pre, f"hT{l}", True)

            fis_pre = {}
            l7 = lmax - 1
            par_chunk = LEAF_CHUNK // 4            # 256

            def emit_l7(j, h8, c8):
                level_chunk(par_chunk, offs[l7] + j * par_chunk, h8, c8,
                            LEAF_CHUNK, 0, hst[l7], cst[l7], m[l7],
                            j * par_chunk, f"L{l7}_{j}",
                            f32mode=(l7 <= PRECISE_LMAX))

            # software pipeline: emit leaf chunk j+1 before the level-7 call
            # that consumes chunk j, so PE has independent work while the
            # level call waits on chunk j's ACT/Pool chain
            l6 = lmax - 2
            step6 = min(m[l6], LPC)
            n6 = m[l6] // step6

            def emit_l6(j6):
                level_chunk(step6, offs[l6] + j6 * step6, hst[l7], cst[l7],
                            m[l7], 4 * j6 * step6, hst[l6], cst[l6], m[l6],
                            j6 * step6, f"L{l6}_{j6}",
                            f32mode=(l6 <= PRECISE_LMAX))

            # pipeline: leaf(j+1) emitted before L7(j); each L6 chunk emitted
            # as soon as the L7 calls covering its children are in
            l7_per_l6 = (4 * step6) // par_chunk
            pend = []
            l7_done = 0
            l6_done = 0
            for j in range(n_chunks):
                h8 = leafp.tile([P, 2 * LEAF_CHUNK], f32r, name=f"h8_{j}",
                                tag="h8")
                c8 = leafp.tile([P, 2 * LEAF_CHUNK], f32, name=f"c8_{j}",
                                tag="c8")
                leaf_chunk(j, h8, c8)
                if j == 0:
                    emit_hoisted_fi()
                pend.append((j, h8, c8))
                if len(pend) > 2:
                    jj, ph, pc = pend.pop(0)
                    emit_l7(jj, ph, pc)
                    l7_done += 1
            for (jj, ph, pc) in pend:
                emit_l7(jj, ph, pc)
                l7_done += 1
            while l6_done < n6:
                emit_l6(l6_done)
                l6_done += 1

            for l in range(lmax - 3, SPLIT_LEVEL - 1, -1):
                step = min(m[l], LPC)
                for j in range(0, m[l], step):
                    pre = [t[:, offs[l] + j:offs[l] + j + step]
                           for t in xsmall]
                    level_chunk(step, offs[l] + j, hst[l + 1], cst[l + 1],
                                m[l + 1], 4 * j, hst[l], cst[l], m[l], j,
                                f"L{l}_{j}", f32mode=(l <= PRECISE_LMAX),
                                xt=pre, fis=fis_pre[l], small=True)

            # ---- local level-2 (children = this core's 8 level-3) ----
            pre2 = [t[:, n_top:n_top + 2] for t in xtop]
            level_chunk(2, 0, hst[SPLIT_LEVEL], cst[SPLIT_LEVEL],
                        m[SPLIT_LEVEL], 0, h2l, c2l, 2, 0, "L2",
                        f32mode=True, xt=pre2, fis=fis_pre[("L2", 0)],
                        small=True)

            # ---- AllGather the 16 level-2 states (2 per core) ----
            blk = P * 2 * 2                    # 512 floats per tensor
            ag_in = dram.tile([1, 2 * blk], f32, name="ag_in")
            ag_out = dram.tile([N_CORES, 2 * blk], f32, name="ag_out")
            nc.gpsimd.dma_start(
                ag_in[:, 0:blk].rearrange("o (p c) -> (o p) c", p=P),
                h2l[:])
            nc.gpsimd.dma_start(
                ag_in[:, blk:2 * blk].rearrange("o (p c) -> (o p) c", p=P),
                c2l[:])
            if timing:
                for g in range(N_CORES):
                    nc.sync.dma_start(ag_out[g:g + 1, :], ag_in[:])
            else:
                nc.gpsimd.collective_compute(
                    "AllGather", mybir.AluOpType.bypass,
                    replica_groups=[list(range(N_CORES))],
                    ins=[ag_in[:].opt()], outs=[ag_out[:].opt()])
            # h2g/c2g: [P, 2, 16] with col = 2*g + c
            nc.gpsimd.dma_start(
                h2g[:].rearrange("p (b g c) -> p b g c", b=2, g=N_CORES),
                ag_out[:, 0:blk]
                .rearrange("g (p b c) -> p b g c", p=P, b=2))
            nc.gpsimd.dma_start(
                c2g[:].rearrange("p (b g c) -> p b g c", b=2, g=N_CORES),
                ag_out[:, blk:2 * blk]
                .rearrange("g (p b c) -> p b g c", p=P, b=2))

            # ---- top levels 1, 0 (replicated) ----
            for l in range(SPLIT_LEVEL - 2, -1, -1):
                cnt = 4**l if l > 0 else 2
                ch_h = h2g if l == SPLIT_LEVEL - 2 else hst[l + 1]
                ch_c = c2g if l == SPLIT_LEVEL - 2 else cst[l + 1]
                mch = 16 if l == SPLIT_LEVEL - 2 else top_cols[l + 1]
                x0 = (4**l - 1) // 3
                pre = [t[:, x0:x0 + cnt] for t in xtop]
                level_chunk(cnt, x0, ch_h, ch_c, mch, 0, hst[l], cst[l],
                            top_cols[l], 0, f"T{l}", f32mode=True, xt=pre,
                            fis=fis_pre[("T", l)], small=True)

            for mt in range(2):
                nc.sync.dma_start(h0_d[:, mt:mt + 1],
                                  b2(hst[0])[:, mt, 0:1])
                nc.sync.dma_start(c0_d[:, mt:mt + 1],
                                  b2(cst[0])[:, mt, 0:1])

    nc.compile()
    return nc


# ---------------------------------------------------------------------------
# self-contained entry point: kernel(**inputs) -> (h[0], c[0])
# ---------------------------------------------------------------------------
N_NODES = 65536

_CACHE = {}


def _ensure_paths():
    import sys
    for p in ("/opt/trn_rl_repo",):
        if p not in sys.path:
            sys.path.insert(0, p)


def _get_runner():
    """Compile the Bass program once and build a reusable jitted SPMD
    executor over the 8 axon-tunneled NeuronCores."""
    if "runner" in _CACHE:
        return _CACHE["runner"]
    _ensure_paths()
    import jax
    from jax.sharding import Mesh, PartitionSpec, NamedSharding
    from jax.experimental.shard_map import shard_map
    from concourse import bass2jax, mybir

    nc = build_program(N_NODES)
    bass2jax.install_neuronx_cc_hook()
    partition_name = (nc.partition_id_tensor.name
                      if nc.partition_id_tensor else None)
    in_names, out_names, out_avals, zero_outs = [], [], [], []
    for alloc in nc.m.functions[0].allocations:
        if not isinstance(alloc, mybir.MemoryLocationSet):
            continue
        name = alloc.memorylocations[0].name
        if alloc.kind == "ExternalInput":
            if name != partition_name:
                in_names.append(name)
        elif alloc.kind == "ExternalOutput":
            out_names.append(name)
            shape = tuple(alloc.tensor_shape)
            dtype = mybir.dt.np(alloc.dtype)
            out_avals.append(jax.core.ShapedArray(shape, dtype))
            zero_outs.append(np.zeros(shape, dtype))
    n_params = len(in_names)
    all_in = list(in_names) + list(out_names)
    if partition_name is not None:
        all_in.append(partition_name)

    def _body(*args):
        operands = list(args)
        if partition_name is not None:
            operands.append(bass2jax.partition_id_tensor())
        return tuple(bass2jax._bass_exec_p.bind(
            *operands, out_avals=tuple(out_avals), in_names=tuple(all_in),
            out_names=tuple(out_names), lowering_input_output_aliases=(),
            sim_require_finite=True, sim_require_nnan=True, nc=nc))

    devices = jax.devices()[:N_CORES]
    assert len(devices) == N_CORES, (
        f"need {N_CORES} neuron devices, found {len(jax.devices())}")
    mesh = Mesh(np.asarray(devices), ("core",))
    nio = n_params + len(out_names)
    sharded = jax.jit(
        shard_map(_body, mesh=mesh,
                  in_specs=(PartitionSpec("core"),) * nio,
                  out_specs=(PartitionSpec("core"),) * len(out_names),
                  check_rep=False),
        keep_unused=True)
    sh = NamedSharding(mesh, PartitionSpec("core"))
    runner = dict(run=sharded, in_names=in_names, out_names=out_names,
                  zero_outs=zero_outs, sh=sh, jax=jax)
    _CACHE["runner"] = runner
    return runner


def kernel(inputs, ix_w, ix_b, ih_w, ih_b, ux_w, ux_b, uh_w, uh_b,
           fi_w, fi_b, fh_w, fh_b):
    """ChildSum TreeLSTM over a complete 4-ary tree of 65536 nodes on 8
    NeuronCores (SPMD, one AllGather at the level-3 frontier).
    Returns (h[0], c[0]) as float32 arrays of shape (256,)."""
    assert np.asarray(inputs).shape == (N_NODES, D)
    in_maps, _ = prep_inputs(
        N_NODES, np.asarray(inputs, np.float32),
        np.asarray(ix_w, np.float32), np.asarray(ix_b, np.float32),
        np.asarray(ih_w, np.float32), np.asarray(ih_b, np.float32),
        np.asarray(ux_w, np.float32), np.asarray(ux_b, np.float32),
        np.asarray(uh_w, np.float32), np.asarray(uh_b, np.float32),
        np.asarray(fi_w, np.float32), np.asarray(fi_b, np.float32),
        np.asarray(fh_w, np.float32), np.asarray(fh_b, np.float32))
    r = _get_runner()
    jax = r["jax"]
    concat = [np.concatenate([in_maps[c][nm] for c in range(N_CORES)], axis=0)
              for nm in r["in_names"]]
    dev_in = [jax.device_put(a, r["sh"]) for a in concat]
    dev_zero = [jax.device_put(
        np.zeros((N_CORES * z.shape[0], *z.shape[1:]), z.dtype), r["sh"])
        for z in r["zero_outs"]]
    outs = r["run"](*dev_in, *dev_zero)
    res = {nm: np.asarray(outs[i]).reshape(N_CORES, P, 2)[0]
           for i, nm in enumerate(r["out_names"])}
    h0 = res["h0"].T.reshape(2 * P).astype(np.float32)
    c0 = res["c0"].T.reshape(2 * P).astype(np.float32)
    return h0, c0

